# revision 20
# baseline (speedup 1.0000x reference)
"""Multi-head self-attention Trainium2 kernel (B=4, S=2048, D=1024, H=16, dk=64).

Sharding (8 cores): data-parallel over batch (4) x tensor-parallel over head
groups (2).  Core c handles batch c//2 and heads [8*(c%2), 8*(c%2)+8), i.e.
feature columns [512*(c%2), 512*(c%2)+512) of Wq/Wk/Wv (column split) and the
matching rows of Wo (row split).  Each core emits a partial [2048, 1024]
output; the host sums the two partials per batch and adds bo.

v2 layout/schedule notes (vs the v1 serial-phase kernel):
  - All PE operands in bf16 (halves DMA + SBUF; PE rate unchanged; measured
    end-to-end rel-err ~5e-3 vs the 2e-2 budget).
  - K bias dropped entirely: q.bk is constant per query row, so it cancels
    in softmax; only Q keeps its (pre-scaled) bias and V keeps bv.
  - Single interleaved instruction stream: attention (phase B) is
    ACT(exp)-bound, so projection (A) and output-projection (C) matmuls are
    emitted between attention kt-steps to keep the PE busy during B and the
    ACT busy as early as possible.  Deps: K(t)/Q(t,qc) before B(qc,t);
    V(t,st) just before its first EV use inside B(qc=0,t); C(qc) after
    B(qc, all t), spread into B(qc+1).
  - scoresT = K_h Q_h^T per head pair uses PE row-tiling (contraction 64,
    base partitions 0/64) so the two matmuls can overlap on hardware.
  - 1/sqrt(dk) folded into Wq/bq on the host.  exp with no max-subtraction
    (scores ~N(0,1)).  Softmax denominator via the ones-column of V65.
"""

import numpy as np
import ml_dtypes

import concourse.bass as bass
import concourse.mybir as mybir
import concourse.tile as tile
from concourse import bacc
from concourse.bass_utils import run_bass_kernel_spmd

F32 = mybir.dt.float32
BF16 = mybir.dt.bfloat16
I16 = mybir.dt.int16

# DVE 1-instruction approx exp: bits_bf16(2^(s*log2e)) ~= round(s*K1 + K2)
# computed fp32-internally by tensor_scalar, output-converted to int16, and
# the int16 bits reinterpreted as bf16.  K2 is tuned so the mean multiplica-
# tive error vs true exp is 1.0 (the sawtooth then has ~1.8% rms, ~4% max,
# and the softmax denominator -- summed from the same approx values via the
# V65 ones column -- keeps the distribution normalized).
EXP_K1 = 128 * 1.4426950408889634
EXP_K2 = 16256.0 - 7.365
# kt steps per b_group whose exp runs on the DVE instead of ACT (5/16 of
# the exp work; ACT keeps the rest + the C-group psum->SBUF copies).
DVE_KT = (2, 5, 8, 11, 14)

P = 128
D = 1024          # model dim
S = 2048          # sequence length
FH = 512          # local feature width (8 heads x 64)
H_LOC = 8         # heads per core
DK = 64           # head dim
N_DT = D // P     # 8 d-tiles
N_FT = FH // P    # 4 local feature tiles == head pairs
N_ST = S // P     # 16 sequence tiles
N_SC = S // 512   # 4 sequence chunks of 512
QC = 512          # query chunk


def _emit(nc, tc, xT, wq, bqc, wk, wv, bv, wo, out):
    Exp = mybir.ActivationFunctionType.Exp
    Add = mybir.AluOpType.add
    Mult = mybir.AluOpType.mult

    from contextlib import ExitStack
    with ExitStack() as es:
        consts = es.enter_context(tc.tile_pool(name="consts", bufs=1))
        persist = es.enter_context(tc.tile_pool(name="persist", bufs=1))
        e_pool = es.enter_context(tc.tile_pool(name="e_pool", bufs=6))
        r_pool = es.enter_context(tc.tile_pool(name="r_pool", bufs=6))
        o_pool = es.enter_context(tc.tile_pool(name="o_pool", bufs=6))
        psA = es.enter_context(tc.tile_pool(name="psA", bufs=2, space="PSUM"))
        psSC = es.enter_context(
            tc.tile_pool(name="psSC", bufs=2, space="PSUM"))
        psEV = es.enter_context(
            tc.tile_pool(name="psEV", bufs=2, space="PSUM"))
        # x/weight pools are opened LAST (top of the SBUF stack) and closed
        # right after their final use (group 13) so a following kernel body
        # can begin its input DMAs while this body drains.
        es_aw = es.enter_context(ExitStack())
        xt_pool = es_aw.enter_context(tc.tile_pool(name="xt_pool", bufs=1))
        w_pool = es_aw.enter_context(tc.tile_pool(name="w_pool", bufs=1))

        ones = consts.tile([1, QC], BF16, tag="ones")
        nc.vector.memset(ones, 1.0)
        # Q bias as a per-partition-scalar column [128, t]: folded into the
        # QT psum->SBUF copy as a tensor_scalar add (frees a PE rank-1
        # matmul per q_group).
        bqc_sb = consts.tile([P, N_FT], F32, tag="bqc")
        nc.scalar.dma_start(out=bqc_sb, in_=bqc[:, :])
        bv_sb = consts.tile([1, FH], F32, tag="bv")
        nc.scalar.dma_start(out=bv_sb, in_=bv[:, :])
        # V bias varies along the free (feature) dim and is constant across
        # sequence partitions: replicate once on gpsimd, then fold into the
        # V65 psum->SBUF copy as a tensor_tensor add.
        bv_rep = consts.tile([P, FH], F32, tag="bv_rep")
        nc.gpsimd.partition_broadcast(bv_rep, bv_sb)

        QT = persist.tile([P, N_FT, S], BF16, tag="QT")
        KT = persist.tile([P, N_FT, S], BF16, tag="KT")
        AO = persist.tile([P, N_FT, S], BF16, tag="AO")
        V65 = persist.tile([P, N_ST, H_LOC, DK + 1], BF16, tag="V65")
        wo_sb = persist.tile([P, N_FT, D], BF16, tag="wo")
        nc.vector.memset(V65[:, :, :, DK:DK + 1], 1.0)

        # Weight/x streams, split per d-tile and interleaved in first-use
        # order so the K(0) accumulation can chase the arriving stream: the
        # phase-B prefix needs (wk_dt, xt_dt) pairs; V fillers need wv;
        # Q(0,0) needs wq.
        wk_sb = w_pool.tile([P, N_FT, N_DT, P], BF16, tag="wk")
        wv_sb = w_pool.tile([P, N_DT, FH], BF16, tag="wv")
        wq_sb = w_pool.tile([P, N_FT, N_DT, P], BF16, tag="wq")
        xt_all = xt_pool.tile([P, N_DT, S], BF16, tag="xt", name="xt_all")
        xts = [xt_all[:, dt, :] for dt in range(N_DT)]
        # The DMA engines drain transfers in issue order (~350GB/s), so
        # order the stream by first-use and keep each prerequisite small:
        # x^T in sc-chunks (1MB), weights in host-pre-swizzled t-chunks
        # (256KB, fully contiguous so no small-line penalty).  The first
        # scores matmul needs only wk-t0 + x-sc0 + wq-t0.  Critical chunks
        # ride the SP queue (the ACT queue starts ~3us late behind the
        # activation-table load).
        nc.sync.dma_start(out=wk_sb[:, 0], in_=wk[0])
        nc.scalar.dma_start(out=wq_sb[:, 0], in_=wq[0])
        # wv on the ACT queue right behind wq0: the first inline V group
        # (g1 slot 0, ~6us in) needs it.
        nc.scalar.dma_start(
            out=wv_sb, in_=wv[:, :].rearrange("(dt p) f -> p dt f", p=P))
        # x^T host-pre-swizzled to [sc, p, dt, s'] so each 1MB sc-chunk is
        # one contiguous-source transfer (>=1KB lines both sides --
        # descriptor-safe on HW): the first scores matmul gates on
        # wk-t0 + x-sc0 + wq-t0 (~1.5MB) instead of all of x^T.
        for sc in range(N_SC):
            nc.sync.dma_start(
                out=xt_all[:, :, sc * QC:(sc + 1) * QC], in_=xT[sc])
        for t in range(1, N_FT):
            nc.scalar.dma_start(out=wk_sb[:, t], in_=wk[t])
            nc.scalar.dma_start(out=wq_sb[:, t], in_=wq[t])
        nc.scalar.dma_start(
            out=wo_sb, in_=wo[:, :].rearrange("(ft p) e -> p ft e", p=P))

        # PE warm-up: junk rank-1 matmuls (no data deps) during the DMA
        # prefix so the HAM clock gate reaches 2.4 GHz before real work.
        for i in range(5):
            wps = psA.tile([P, QC], F32, tag="psA", name="wps")
            for j in range(2):
                nc.tensor.matmul(
                    wps, ones[:, 0:P], ones, start=(j == 0), stop=(j == 1))

        # ---------------- emission helpers ----------------
        def v_group(st):
            # V for all 8 heads, rows of sequence tile st, natural [s, f]
            # layout.  512-wide moving operand (the t-blocks of wv_sb are a
            # strided free dim).  bv rides the psum->SBUF copy (DVE add).
            ps = psA.tile([P, QC], F32, tag="psA", name="psv")
            for dt in range(N_DT):
                nc.tensor.matmul(
                    ps,
                    xts[dt][:, st * P:(st + 1) * P],
                    wv_sb[:, dt, :],
                    start=(dt == 0), stop=(dt == N_DT - 1))
            nc.vector.tensor_add(
                out=V65[:, st, :, 0:DK],
                in0=ps.rearrange("p (h d) -> p h d", h=H_LOC),
                in1=bv_rep.rearrange("p (h d) -> p h d", h=H_LOC))

        def k_group(t, sc):
            ps = psA.tile([P, QC], F32, tag="psA", name="psk")
            for dt in range(N_DT):
                nc.tensor.matmul(
                    ps,
                    wk_sb[:, t, dt, :],
                    xts[dt][:, sc * QC:(sc + 1) * QC],
                    start=(dt == 0), stop=(dt == N_DT - 1))
            nc.vector.tensor_copy(
                out=KT[:, t, sc * QC:(sc + 1) * QC], in_=ps)

        def q_group(t, sc):
            ps = psA.tile([P, QC], F32, tag="psA", name="psq")
            for dt in range(N_DT):
                nc.tensor.matmul(
                    ps,
                    wq_sb[:, t, dt, :],
                    xts[dt][:, sc * QC:(sc + 1) * QC],
                    start=(dt == 0), stop=(dt == N_DT - 1))
            # bias as per-partition scalar, fused into the copy.
            nc.vector.tensor_scalar(
                out=QT[:, t, sc * QC:(sc + 1) * QC], in0=ps,
                scalar1=bqc_sb[:, t:t + 1], scalar2=None, op0=Add)

        def c_group(st, ec, on_act=False):
            ps = psA.tile([P, QC], F32, tag="psA", name="psc")
            for ft in range(N_FT):
                nc.tensor.matmul(
                    ps,
                    AO[:, ft, st * P:(st + 1) * P],
                    wo_sb[:, ft, ec * QC:(ec + 1) * QC],
                    start=(ft == 0), stop=(ft == N_FT - 1))
            ob = o_pool.tile([P, QC], F32, tag="ob", name="ob")
            # All C psum->SBUF copies ride ACT (DVE carries the offloaded
            # exp work).  Tail groups additionally issue their out-DMA from
            # the ACT HWDGE queue so the final transfers drain two queues
            # wide instead of serializing on SP.
            nc.scalar.copy(out=ob, in_=ps)
            if on_act:
                nc.scalar.dma_start(
                    out=out[st * P:(st + 1) * P, ec * QC:(ec + 1) * QC],
                    in_=ob)
            else:
                nc.sync.dma_start(
                    out=out[st * P:(st + 1) * P, ec * QC:(ec + 1) * QC],
                    in_=ob)

        def b_group(qc, t, fillers):
            # fillers: list of 16 lists of closures, one list per kt step.
            # The kt loop is software-pipelined one step: EV(kt-1) is
            # emitted after scores(kt)/exp(kt) so the PE does not wait on
            # the ACT latency of the exp it is about to consume.  The
            # group's last EV + normalize stay at group end (deferring them
            # into the next group raced that group's bank-clearing
            # start=True and produced NaN on hardware).
            ev = [psEV.tile([DK + 1, QC], F32, tag="ev", name=f"ev{h2}")
                  for h2 in range(2)]
            es_tiles = []

            def ev_step(kt):
                e = es_tiles[kt]
                for h2 in range(2):
                    nc.tensor.matmul(
                        ev[h2],
                        V65[:, kt, 2 * t + h2, :],
                        e[:, h2 * QC:(h2 + 1) * QC],
                        start=(kt == 0), stop=(kt == N_ST - 1),
                        skip_group_check=True)

            for kt in range(N_ST):
                if qc == 0 and kt in (3, 6, 10):
                    k_group(t, {3: 1, 6: 2, 10: 3}[kt])
                if qc == 0 and t == 0:
                    v_group(st=kt)
                for f in fillers[kt]:
                    f()
                ps = psSC.tile([P, 2 * QC], F32, tag="sc", name="scps")
                for h2 in range(2):
                    lo = h2 * DK
                    nc.tensor.matmul(
                        ps[:, h2 * QC:(h2 + 1) * QC],
                        KT[lo:lo + DK, t, kt * P:(kt + 1) * P],
                        QT[lo:lo + DK, t, qc * QC:(qc + 1) * QC],
                        start=True, stop=True,
                        skip_group_check=True)
                ei = e_pool.tile([P, 2 * QC], I16, tag="e", name="esb")
                e = ei.bitcast(BF16)
                if kt in DVE_KT:
                    # approx exp on DVE: one tensor_scalar producing the
                    # bf16 bit pattern of 2^(s*log2e) as int16.
                    nc.vector.tensor_scalar(
                        out=ei, in0=ps, scalar1=EXP_K1, scalar2=EXP_K2,
                        op0=Mult, op1=Add)
                else:
                    nc.scalar.activation(out=e, in_=ps, func=Exp)
                es_tiles.append(e)
                if kt > 0:
                    ev_step(kt - 1)
            ev_step(N_ST - 1)
            last = (qc == N_SC - 1 and t == N_FT - 1)
            for h2 in range(2):
                # The custom-DVE approx reciprocal mishandles mismatched
                # in/out base partitions, so stage the denominator row to
                # partition 0 with a plain copy first (~0.66us vs the 3.2us
                # single-lane exact reciprocal).
                r1d = r_pool.tile([1, QC], F32, tag="r1d", name="r1d")
                nc.vector.tensor_copy(out=r1d, in_=ev[h2][DK:DK + 1, :])
                r1 = r_pool.tile([1, QC], F32, tag="r1", name="r1")
                nc.vector.reciprocal_approx_fast(out=r1, in_=r1d)
                rb = r_pool.tile([DK, QC], F32, tag="rb", name="rb")
                nc.gpsimd.partition_broadcast(rb, r1)
                # For the final group, write AO in 128-column chunks so the
                # tail C groups (which read one st = 128 columns each)
                # unblock as soon as their chunk lands.
                n_chunks = 4 if last else 1
                w = QC // n_chunks
                for c in range(n_chunks):
                    nc.vector.tensor_mul(
                        out=AO[h2 * DK:(h2 + 1) * DK, t,
                               qc * QC + c * w:qc * QC + (c + 1) * w],
                        in0=ev[h2][0:DK, c * w:(c + 1) * w],
                        in1=rb[:, c * w:(c + 1) * w])

        # ---------------- schedule ----------------
        # Staircase (anti-diagonal) order over (qc, t): spreads phase-A
        # emission across the first ~10 B groups (instead of cramming it
        # into qc=0) so no segment is ACT-bound while another is PE-bound.
        # Within a diagonal t ascends, delaying each t's first use (max
        # slack for A(t)); C(qc) unlocks after B(qc, t=3) and fills the
        # late groups.
        order = [(0, 0), (1, 0), (0, 1), (2, 0), (1, 1), (0, 2), (3, 0),
                 (2, 1), (1, 2), (0, 3), (3, 1), (2, 2), (1, 3), (3, 2),
                 (2, 3), (3, 3)]

        def K(t, sc):
            return lambda: k_group(t, sc)

        def Q(t, sc):
            return lambda: q_group(t, sc)

        def C(st, ec):
            return lambda: c_group(st, ec)

        fill_plan = {
            1: [Q(0, 1), K(1, 0), Q(1, 0)],
            2: [Q(0, 2)],
            3: [Q(1, 1), K(2, 0), Q(2, 0)],
            4: [Q(0, 3)],
            5: [Q(1, 2)],
            6: [Q(2, 1), K(3, 0), Q(3, 0)],
            7: [Q(1, 3)],
            8: [Q(2, 2)],
            9: [Q(3, 1)],
            11: [C(0, 0), C(0, 1), C(1, 0)],
            12: [C(1, 1), C(2, 0), C(2, 1), Q(2, 3)],
            13: [C(3, 0), C(3, 1), Q(3, 2), Q(3, 3)],
            14: [C(4, 0), C(4, 1), C(5, 0), C(5, 1)],
            15: [C(6, 0), C(6, 1), C(7, 0), C(7, 1)],
            16: [C(8, 0), C(8, 1), C(9, 0), C(9, 1),
                 C(10, 0), C(10, 1), C(11, 0), C(11, 1)],
        }

        # Prefix: just K(0,0) + Q(0,0); K(0,sc>=1) ride inside B(0,0) as
        # the x^T sc-chunks land.
        k_group(0, 0)
        q_group(0, 0)

        for gi, (qc, t) in enumerate(order, start=1):
            slots = [[] for _ in range(N_ST)]
            base = 11 if qc == 0 else 5
            for i, f in enumerate(fill_plan.get(gi, [])):
                slots[(base + 2 * i) % N_ST].append(f)
            b_group(qc, t, slots)
            if gi == 13:
                es_aw.close()

        # Tail: C(qc=3) sts; psum->SBUF copies alternate ACT/DVE.
        for st in range(4 * (N_SC - 1), N_ST):
            for ec in range(D // QC):
                c_group(st, ec, on_act=(ec == 0))


def build_nc(debug=False, repeat=1):
    nc = bacc.Bacc("TRN2", debug=debug)
    xT = nc.declare_dram_parameter("xT", [N_SC, P, N_DT, QC], BF16,
                                   isOutput=False)
    wq = nc.declare_dram_parameter("wq", [N_FT, P, N_DT, P], BF16,
                                   isOutput=False)
    bqc = nc.declare_dram_parameter("bqc", [P, N_FT], F32, isOutput=False)
    wk = nc.declare_dram_parameter("wk", [N_FT, P, N_DT, P], BF16,
                                   isOutput=False)
    wv = nc.declare_dram_parameter("wv", [D, FH], BF16, isOutput=False)
    bv = nc.declare_dram_parameter("bv", [1, FH], F32, isOutput=False)
    wo = nc.declare_dram_parameter("wo", [FH, D], BF16, isOutput=False)
    out = nc.declare_dram_parameter("out", [S, D], F32, isOutput=True)
    with tile.TileContext(nc) as tc:
        for _rep in range(repeat):
            _emit(nc, tc, xT[:, :], wq[:, :], bqc[:, :], wk[:, :],
                  wv[:, :], bv[:, :], wo[:, :], out[:, :])
    nc.compile()
    return nc


def _bf16(a):
    return np.ascontiguousarray(np.asarray(a, np.float32)).astype(
        ml_dtypes.bfloat16)


def _swz(w):
    """[D, FH] -> [t, p, dt, f]: element (t,p,dt,f) = w[dt*128+p, t*128+f]."""
    return np.ascontiguousarray(
        np.asarray(w, np.float32).reshape(N_DT, P, N_FT, P)
        .transpose(2, 1, 0, 3)).astype(ml_dtypes.bfloat16)


def make_in_maps(x, Wq, bq, Wk, bk, Wv, bv, Wo):
    del bk  # q.bk is constant per query row -> cancels in softmax
    in_maps = []
    for c in range(8):
        b, hg = divmod(c, 2)
        F = slice(FH * hg, FH * (hg + 1))
        in_maps.append({
            "xT": _bf16(np.ascontiguousarray(
                x[b].T.reshape(N_DT, P, N_SC, QC).transpose(2, 1, 0, 3))),
            "wq": _swz(Wq[:, F] * 0.125),
            "bqc": np.ascontiguousarray(
                (bq[F] * 0.125).reshape(N_FT, P).T).astype(np.float32),
            "wk": _swz(Wk[:, F]),
            "wv": _bf16(Wv[:, F]),
            "bv": np.asarray(bv[F].reshape(1, FH), np.float32),
            "wo": _bf16(Wo[F, :]),
        })
    return in_maps


_NC_CACHE = None


def _get_nc():
    global _NC_CACHE
    if _NC_CACHE is None:
        _NC_CACHE = build_nc()
    return _NC_CACHE


def kernel(x, Wq, bq, Wk, bk, Wv, bv, Wo, bo, _trace=False):
    x = np.asarray(x, np.float32)
    args = [np.asarray(a, np.float32) for a in (Wq, bq, Wk, bk, Wv, bv, Wo)]
    bo = np.asarray(bo, np.float32)
    nc = _get_nc()
    in_maps = make_in_maps(x, *args)
    res = run_bass_kernel_spmd(nc, in_maps, list(range(8)), trace=_trace)
    out = np.empty((4, S, D), np.float32)
    for b in range(4):
        out[b] = res.results[2 * b]["out"] + res.results[2 * b + 1]["out"] + bo
    if _trace:
        return out, res
    return out



# revision 33
# speedup vs baseline: 1.3536x; 1.3536x over previous
"""Multi-head self-attention Trainium2 kernel (B=4, S=2048, D=1024, H=16, dk=64).

Sharding (8 cores): data-parallel over batch (4) x tensor-parallel over head
groups (2).  Core c handles batch c//2 and heads [8*(c%2), 8*(c%2)+8), i.e.
feature columns [512*(c%2), 512*(c%2)+512) of Wq/Wk/Wv (column split) and the
matching rows of Wo (row split).  Each core emits a partial [2048, 1024]
output; the host sums the two partials per batch and adds bo.

v2 layout/schedule notes (vs the v1 serial-phase kernel):
  - All PE operands in bf16 (halves DMA + SBUF; PE rate unchanged).
  - K bias dropped entirely: q.bk is constant per query row, so it cancels
    in softmax; only Q keeps its (pre-scaled) bias and V keeps bv.
  - Single interleaved instruction stream: projection (A) and
    output-projection (C) matmuls are emitted between attention kt-steps so
    no engine idles during phase B.  Deps: K(t)/Q(t,qc) before B(qc,t);
    V(t,st) just before its first EV use inside B(qc=0,t); C(qc) after
    B(qc, all t), spread into B(qc+1).
  - scoresT = K_h Q_h^T per head pair uses PE row-tiling (contraction 64,
    base partitions 0/64) so the two matmuls overlap on hardware (verified:
    forcing both onto tile (0,0) costs ~15%).
  - 1/sqrt(dk) folded into Wq/bq on the host.  exp with no max-subtraction
    (scores ~N(0,1)).  Softmax denominator via the ones-column of V65.

v3 changes (measured ~1.3x vs v2 on interleaved A/B at repeat=13/25):
  - 5/16 of the exp tiles run on the DVE as a one-instruction Schraudolph
    approx exp (tensor_scalar mult+add, fp32-internal, int16 output whose
    bits are the bf16 pattern of 2^(s*log2e); K2 tuned for unit mean error;
    the sawtooth adds ~0.5% end-to-end, rel-err 6.9e-3 vs 2e-2 budget).
  - Q/V bias matmuls folded into the psum->SBUF evacuations (tensor_scalar
    per-partition add / tensor_tensor add with a gpsimd-replicated row).
  - Softmax normalize: denominator at psum partition 0 (ones column FIRST
    in the padded EV stationary; V at columns 64..127), read directly by
    the custom-DVE reciprocal_approx_fast (which only works at base
    partition 0; 0.7us vs 3.2us for the single-lane exact reciprocal).
  - EV software-pipeline lag raised 1 -> 3 kt steps (PE stalls less on the
    exp latency).
  - C psum->SBUF copies on ACT; output stored bf16 (halves the out-DMA).
"""

import numpy as np
import ml_dtypes

import concourse.bass as bass
import concourse.mybir as mybir
import concourse.tile as tile
from concourse import bacc
from concourse.bass_utils import run_bass_kernel_spmd

F32 = mybir.dt.float32
BF16 = mybir.dt.bfloat16
I16 = mybir.dt.int16

# DVE 1-instruction approx exp: bits_bf16(2^(s*log2e)) ~= round(s*K1 + K2)
# computed fp32-internally by tensor_scalar, output-converted to int16, and
# the int16 bits reinterpreted as bf16.  K2 is tuned so the mean multiplica-
# tive error vs true exp is 1.0 (the sawtooth then has ~1.8% rms, ~4% max,
# and the softmax denominator -- summed from the same approx values via the
# V65 ones column -- keeps the distribution normalized).
EXP_K1 = 128 * 1.4426950408889634
EXP_K2 = 16256.0 - 7.365
# kt steps per b_group whose exp runs on the DVE instead of ACT (5/16 of
# the exp work; ACT keeps the rest + the C-group psum->SBUF copies).
DVE_KT = (2, 5, 8, 11, 14)

P = 128
D = 1024          # model dim
S = 2048          # sequence length
FH = 512          # local feature width (8 heads x 64)
H_LOC = 8         # heads per core
DK = 64           # head dim
N_DT = D // P     # 8 d-tiles
N_FT = FH // P    # 4 local feature tiles == head pairs
N_ST = S // P     # 16 sequence tiles
N_SC = S // 512   # 4 sequence chunks of 512
QC = 512          # query chunk


def _emit(nc, tc, xT, wq, bqc, wk, wv, bv, wo, out):
    Exp = mybir.ActivationFunctionType.Exp
    Add = mybir.AluOpType.add
    Mult = mybir.AluOpType.mult

    from contextlib import ExitStack
    with ExitStack() as es:
        consts = es.enter_context(tc.tile_pool(name="consts", bufs=1))
        persist = es.enter_context(tc.tile_pool(name="persist", bufs=1))
        e_pool = es.enter_context(tc.tile_pool(name="e_pool", bufs=6))
        r_pool = es.enter_context(tc.tile_pool(name="r_pool", bufs=6))
        o_pool = es.enter_context(tc.tile_pool(name="o_pool", bufs=6))
        psA = es.enter_context(tc.tile_pool(name="psA", bufs=2, space="PSUM"))
        psSC = es.enter_context(
            tc.tile_pool(name="psSC", bufs=2, space="PSUM"))
        psEV = es.enter_context(
            tc.tile_pool(name="psEV", bufs=2, space="PSUM"))
        # x/weight pools are opened LAST (top of the SBUF stack) and closed
        # right after their final use (group 13) so a following kernel body
        # can begin its input DMAs while this body drains.
        es_aw = es.enter_context(ExitStack())
        xt_pool = es_aw.enter_context(tc.tile_pool(name="xt_pool", bufs=1))
        w_pool = es_aw.enter_context(tc.tile_pool(name="w_pool", bufs=1))

        ones = consts.tile([1, QC], BF16, tag="ones")
        nc.vector.memset(ones, 1.0)
        # Q bias as a per-partition-scalar column [128, t]: folded into the
        # QT psum->SBUF copy as a tensor_scalar add (frees a PE rank-1
        # matmul per q_group).
        bqc_sb = consts.tile([P, N_FT], F32, tag="bqc")
        nc.scalar.dma_start(out=bqc_sb, in_=bqc[:, :])
        bv_sb = consts.tile([1, FH], F32, tag="bv")
        nc.scalar.dma_start(out=bv_sb, in_=bv[:, :])
        # V bias varies along the free (feature) dim and is constant across
        # sequence partitions: replicate once on gpsimd, then fold into the
        # V65 psum->SBUF copy as a tensor_tensor add.
        bv_rep = consts.tile([P, FH], F32, tag="bv_rep")
        nc.gpsimd.partition_broadcast(bv_rep, bv_sb)

        QT = persist.tile([P, N_FT, S], BF16, tag="QT")
        KT = persist.tile([P, N_FT, S], BF16, tag="KT")
        AO = persist.tile([P, N_FT, S], BF16, tag="AO")
        # EV stationary padded to the full 128 columns: ones at column 0 (so
        # the softmax denominator lands on psum partition 0, where the
        # base-partition-0-only custom-DVE approx reciprocal can read it
        # straight from PSUM) and V at columns 64..127 (so the AO normalize
        # reads psum[64:128] -- a legal 64-aligned partition window).
        # Columns 1..63 are never read; stationary width is free on the PE.
        V65 = persist.tile([P, N_ST, H_LOC, P], BF16, tag="V65")
        wo_sb = persist.tile([P, N_FT, D], BF16, tag="wo")
        nc.vector.memset(V65[:, :, :, 0:1], 1.0)

        # Weight/x streams, split per d-tile and interleaved in first-use
        # order so the K(0) accumulation can chase the arriving stream: the
        # phase-B prefix needs (wk_dt, xt_dt) pairs; V fillers need wv;
        # Q(0,0) needs wq.
        wk_sb = w_pool.tile([P, N_FT, N_DT, P], BF16, tag="wk")
        wv_sb = w_pool.tile([P, N_DT, FH], BF16, tag="wv")
        wq_sb = w_pool.tile([P, N_FT, N_DT, P], BF16, tag="wq")
        xt_all = xt_pool.tile([P, N_DT, S], BF16, tag="xt", name="xt_all")
        xts = [xt_all[:, dt, :] for dt in range(N_DT)]
        # The DMA engines drain transfers in issue order (~350GB/s), so
        # order the stream by first-use and keep each prerequisite small:
        # x^T in sc-chunks (1MB), weights in host-pre-swizzled t-chunks
        # (256KB, fully contiguous so no small-line penalty).  The first
        # scores matmul needs only wk-t0 + x-sc0 + wq-t0.  Critical chunks
        # ride the SP queue (the ACT queue starts ~3us late behind the
        # activation-table load).
        nc.sync.dma_start(out=wk_sb[:, 0], in_=wk[0])
        nc.scalar.dma_start(out=wq_sb[:, 0], in_=wq[0])
        # wv on the ACT queue right behind wq0: the first inline V group
        # (g1 slot 0, ~6us in) needs it.
        nc.scalar.dma_start(
            out=wv_sb, in_=wv[:, :].rearrange("(dt p) f -> p dt f", p=P))
        # x^T host-pre-swizzled to [sc, p, dt, s'] so each 1MB sc-chunk is
        # one contiguous-source transfer (>=1KB lines both sides --
        # descriptor-safe on HW): the first scores matmul gates on
        # wk-t0 + x-sc0 + wq-t0 (~1.5MB) instead of all of x^T.
        for sc in range(N_SC):
            nc.sync.dma_start(
                out=xt_all[:, :, sc * QC:(sc + 1) * QC], in_=xT[sc])
        for t in range(1, N_FT):
            nc.scalar.dma_start(out=wk_sb[:, t], in_=wk[t])
            nc.scalar.dma_start(out=wq_sb[:, t], in_=wq[t])
        nc.scalar.dma_start(
            out=wo_sb, in_=wo[:, :].rearrange("(ft p) e -> p ft e", p=P))

        # PE warm-up: one junk rank-1 matmul pair (no data deps) during the
        # DMA prefix so the HAM clock gate starts ramping before real work.
        # (In steady-state repeat the PE stays busy across body boundaries,
        # so more warm-up is pure waste.)
        for i in range(1):
            wps = psA.tile([P, QC], F32, tag="psA", name="wps")
            for j in range(2):
                nc.tensor.matmul(
                    wps, ones[:, 0:P], ones, start=(j == 0), stop=(j == 1))

        # ---------------- emission helpers ----------------
        def v_group(st):
            # V for all 8 heads, rows of sequence tile st, natural [s, f]
            # layout.  512-wide moving operand (the t-blocks of wv_sb are a
            # strided free dim).  bv rides the psum->SBUF copy (DVE add).
            ps = psA.tile([P, QC], F32, tag="psA", name="psv")
            for dt in range(N_DT):
                nc.tensor.matmul(
                    ps,
                    xts[dt][:, st * P:(st + 1) * P],
                    wv_sb[:, dt, :],
                    start=(dt == 0), stop=(dt == N_DT - 1))
            nc.vector.tensor_add(
                out=V65[:, st, :, DK:2 * DK],
                in0=ps.rearrange("p (h d) -> p h d", h=H_LOC),
                in1=bv_rep.rearrange("p (h d) -> p h d", h=H_LOC))

        def k_group(t, sc):
            ps = psA.tile([P, QC], F32, tag="psA", name="psk")
            for dt in range(N_DT):
                nc.tensor.matmul(
                    ps,
                    wk_sb[:, t, dt, :],
                    xts[dt][:, sc * QC:(sc + 1) * QC],
                    start=(dt == 0), stop=(dt == N_DT - 1))
            nc.vector.tensor_copy(
                out=KT[:, t, sc * QC:(sc + 1) * QC], in_=ps)

        def q_group(t, sc):
            ps = psA.tile([P, QC], F32, tag="psA", name="psq")
            for dt in range(N_DT):
                nc.tensor.matmul(
                    ps,
                    wq_sb[:, t, dt, :],
                    xts[dt][:, sc * QC:(sc + 1) * QC],
                    start=(dt == 0), stop=(dt == N_DT - 1))
            # bias as per-partition scalar, fused into the copy.
            nc.vector.tensor_scalar(
                out=QT[:, t, sc * QC:(sc + 1) * QC], in0=ps,
                scalar1=bqc_sb[:, t:t + 1], scalar2=None, op0=Add)

        def c_group(st, ec, on_act=False):
            ps = psA.tile([P, QC], F32, tag="psA", name="psc")
            for ft in range(N_FT):
                nc.tensor.matmul(
                    ps,
                    AO[:, ft, st * P:(st + 1) * P],
                    wo_sb[:, ft, ec * QC:(ec + 1) * QC],
                    start=(ft == 0), stop=(ft == N_FT - 1))
            ob = o_pool.tile([P, QC], BF16, tag="ob", name="ob")
            # All C psum->SBUF copies ride ACT (DVE carries the offloaded
            # exp work).  Tail groups additionally issue their out-DMA from
            # the ACT HWDGE queue so the final transfers drain two queues
            # wide instead of serializing on SP.
            nc.scalar.copy(out=ob, in_=ps)
            if on_act:
                nc.scalar.dma_start(
                    out=out[st * P:(st + 1) * P, ec * QC:(ec + 1) * QC],
                    in_=ob)
            else:
                nc.sync.dma_start(
                    out=out[st * P:(st + 1) * P, ec * QC:(ec + 1) * QC],
                    in_=ob)

        def b_group(qc, t, fillers):
            # fillers: list of 16 lists of closures, one list per kt step.
            # The kt loop is software-pipelined one step: EV(kt-1) is
            # emitted after scores(kt)/exp(kt) so the PE does not wait on
            # the ACT latency of the exp it is about to consume.  The
            # group's last EV + normalize stay at group end (deferring them
            # into the next group raced that group's bank-clearing
            # start=True and produced NaN on hardware).
            ev = [psEV.tile([P, QC], F32, tag="ev", name=f"ev{h2}")
                  for h2 in range(2)]
            es_tiles = []

            def ev_step(kt):
                e = es_tiles[kt]
                for h2 in range(2):
                    nc.tensor.matmul(
                        ev[h2],
                        V65[:, kt, 2 * t + h2, :],
                        e[:, h2 * QC:(h2 + 1) * QC],
                        start=(kt == 0), stop=(kt == N_ST - 1),
                        skip_group_check=True)

            for kt in range(N_ST):
                if qc == 0 and kt in (3, 6, 10):
                    k_group(t, {3: 1, 6: 2, 10: 3}[kt])
                if qc == 0 and t == 0:
                    v_group(st=kt)
                for f in fillers[kt]:
                    f()
                ps = psSC.tile([P, 2 * QC], F32, tag="sc", name="scps")
                for h2 in range(2):
                    lo = h2 * DK
                    nc.tensor.matmul(
                        ps[:, h2 * QC:(h2 + 1) * QC],
                        KT[lo:lo + DK, t, kt * P:(kt + 1) * P],
                        QT[lo:lo + DK, t, qc * QC:(qc + 1) * QC],
                        start=True, stop=True,
                        skip_group_check=True)
                ei = e_pool.tile([P, 2 * QC], I16, tag="e", name="esb")
                e = ei.bitcast(BF16)
                if kt in DVE_KT:
                    # approx exp on DVE: one tensor_scalar producing the
                    # bf16 bit pattern of 2^(s*log2e) as int16.
                    nc.vector.tensor_scalar(
                        out=ei, in0=ps, scalar1=EXP_K1, scalar2=EXP_K2,
                        op0=Mult, op1=Add)
                else:
                    nc.scalar.activation(out=e, in_=ps, func=Exp)
                es_tiles.append(e)
                if kt > 2:
                    ev_step(kt - 3)
            ev_step(N_ST - 3)
            ev_step(N_ST - 2)
            ev_step(N_ST - 1)
            last = (qc == N_SC - 1 and t == N_FT - 1)
            for h2 in range(2):
                # Denominator on psum partition 0 (ones column is V65[...,0]):
                # the custom-DVE approx reciprocal reads it directly (it
                # mishandles mismatched in/out base partitions, but 0->0 is
                # aligned).  ~0.7us vs the 3.2us single-lane exact recip.
                r1 = r_pool.tile([1, QC], F32, tag="r1", name="r1")
                nc.vector.reciprocal_approx_fast(
                    out=r1, in_=ev[h2][0:1, :])
                rb = r_pool.tile([DK, QC], F32, tag="rb", name="rb")
                nc.gpsimd.partition_broadcast(rb, r1)
                # For the final group, write AO in 128-column chunks so the
                # tail C groups (which read one st = 128 columns each)
                # unblock as soon as their chunk lands.
                n_chunks = 4 if last else 1
                w = QC // n_chunks
                for c in range(n_chunks):
                    nc.vector.tensor_mul(
                        out=AO[h2 * DK:(h2 + 1) * DK, t,
                               qc * QC + c * w:qc * QC + (c + 1) * w],
                        in0=ev[h2][DK:2 * DK, c * w:(c + 1) * w],
                        in1=rb[:, c * w:(c + 1) * w])

        # ---------------- schedule ----------------
        # Staircase (anti-diagonal) order over (qc, t): spreads phase-A
        # emission across the first ~10 B groups (instead of cramming it
        # into qc=0) so no segment is ACT-bound while another is PE-bound.
        # Within a diagonal t ascends, delaying each t's first use (max
        # slack for A(t)); C(qc) unlocks after B(qc, t=3) and fills the
        # late groups.
        order = [(0, 0), (1, 0), (0, 1), (2, 0), (1, 1), (0, 2), (3, 0),
                 (2, 1), (1, 2), (0, 3), (3, 1), (2, 2), (1, 3), (3, 2),
                 (2, 3), (3, 3)]

        def K(t, sc):
            return lambda: k_group(t, sc)

        def Q(t, sc):
            return lambda: q_group(t, sc)

        def C(st, ec):
            return lambda: c_group(st, ec)

        fill_plan = {
            1: [Q(0, 1), K(1, 0), Q(1, 0)],
            2: [Q(0, 2)],
            3: [Q(1, 1), K(2, 0), Q(2, 0)],
            4: [Q(0, 3)],
            5: [Q(1, 2)],
            6: [Q(2, 1), K(3, 0), Q(3, 0)],
            7: [Q(1, 3)],
            8: [Q(2, 2)],
            9: [Q(3, 1)],
            11: [C(0, 0), C(0, 1), C(1, 0)],
            12: [C(1, 1), C(2, 0), C(2, 1), Q(2, 3)],
            13: [C(3, 0), C(3, 1), Q(3, 2), Q(3, 3)],
            14: [C(4, 0), C(4, 1), C(5, 0), C(5, 1)],
            15: [C(6, 0), C(6, 1), C(7, 0), C(7, 1)],
            16: [C(8, 0), C(8, 1), C(9, 0), C(9, 1),
                 C(10, 0), C(10, 1), C(11, 0), C(11, 1)],
        }

        # Prefix: just K(0,0) + Q(0,0); K(0,sc>=1) ride inside B(0,0) as
        # the x^T sc-chunks land.
        k_group(0, 0)
        q_group(0, 0)

        for gi, (qc, t) in enumerate(order, start=1):
            slots = [[] for _ in range(N_ST)]
            base = 11 if qc == 0 else 5
            for i, f in enumerate(fill_plan.get(gi, [])):
                slots[(base + 2 * i) % N_ST].append(f)
            b_group(qc, t, slots)
            if gi == 13:
                es_aw.close()

        # Tail: C(qc=3) sts; psum->SBUF copies alternate ACT/DVE.
        for st in range(4 * (N_SC - 1), N_ST):
            for ec in range(D // QC):
                c_group(st, ec, on_act=(ec == 0))


def build_nc(debug=False, repeat=1):
    nc = bacc.Bacc("TRN2", debug=debug)
    xT = nc.declare_dram_parameter("xT", [N_SC, P, N_DT, QC], BF16,
                                   isOutput=False)
    wq = nc.declare_dram_parameter("wq", [N_FT, P, N_DT, P], BF16,
                                   isOutput=False)
    bqc = nc.declare_dram_parameter("bqc", [P, N_FT], F32, isOutput=False)
    wk = nc.declare_dram_parameter("wk", [N_FT, P, N_DT, P], BF16,
                                   isOutput=False)
    wv = nc.declare_dram_parameter("wv", [D, FH], BF16, isOutput=False)
    bv = nc.declare_dram_parameter("bv", [1, FH], F32, isOutput=False)
    wo = nc.declare_dram_parameter("wo", [FH, D], BF16, isOutput=False)
    out = nc.declare_dram_parameter("out", [S, D], BF16, isOutput=True)
    with tile.TileContext(nc) as tc:
        for _rep in range(repeat):
            _emit(nc, tc, xT[:, :], wq[:, :], bqc[:, :], wk[:, :],
                  wv[:, :], bv[:, :], wo[:, :], out[:, :])
    nc.compile()
    return nc


def _bf16(a):
    return np.ascontiguousarray(np.asarray(a, np.float32)).astype(
        ml_dtypes.bfloat16)


def _swz(w):
    """[D, FH] -> [t, p, dt, f]: element (t,p,dt,f) = w[dt*128+p, t*128+f]."""
    return np.ascontiguousarray(
        np.asarray(w, np.float32).reshape(N_DT, P, N_FT, P)
        .transpose(2, 1, 0, 3)).astype(ml_dtypes.bfloat16)


def make_in_maps(x, Wq, bq, Wk, bk, Wv, bv, Wo):
    del bk  # q.bk is constant per query row -> cancels in softmax
    in_maps = []
    for c in range(8):
        b, hg = divmod(c, 2)
        F = slice(FH * hg, FH * (hg + 1))
        in_maps.append({
            "xT": _bf16(np.ascontiguousarray(
                x[b].T.reshape(N_DT, P, N_SC, QC).transpose(2, 1, 0, 3))),
            "wq": _swz(Wq[:, F] * 0.125),
            "bqc": np.ascontiguousarray(
                (bq[F] * 0.125).reshape(N_FT, P).T).astype(np.float32),
            "wk": _swz(Wk[:, F]),
            "wv": _bf16(Wv[:, F]),
            "bv": np.asarray(bv[F].reshape(1, FH), np.float32),
            "wo": _bf16(Wo[F, :]),
        })
    return in_maps


_NC_CACHE = None


def _get_nc():
    global _NC_CACHE
    if _NC_CACHE is None:
        _NC_CACHE = build_nc()
    return _NC_CACHE


def kernel(x, Wq, bq, Wk, bk, Wv, bv, Wo, bo, _trace=False):
    x = np.asarray(x, np.float32)
    args = [np.asarray(a, np.float32) for a in (Wq, bq, Wk, bk, Wv, bv, Wo)]
    bo = np.asarray(bo, np.float32)
    nc = _get_nc()
    in_maps = make_in_maps(x, *args)
    res = run_bass_kernel_spmd(nc, in_maps, list(range(8)), trace=_trace)
    out = np.empty((4, S, D), np.float32)
    for b in range(4):
        out[b] = (res.results[2 * b]["out"].astype(np.float32)
                  + res.results[2 * b + 1]["out"].astype(np.float32) + bo)
    if _trace:
        return out, res
    return out

